# revision 32
# baseline (speedup 1.0000x reference)
"""Trainium2 Bass kernel for causal bilinear self-attention (diagonal variant).

Computes, per (b, head):
    scores[t, s] = h[b, t] @ A[head] @ h[b, s]        (causal: s <= t)
    attn = softmax(scores, axis=-1)
    out[b, head, t, :] = attn[t, t] * h[b, t, :]
returned reshaped row-major to (B, T, H*d)  (faithful torch .view semantics).

Only the diagonal of the attention matrix is needed:
    attn[t, t] = 1 / sum_{s<=t} exp(scores[t,s] - scores[t,t])
Using bias = -scores[t,t] inside the exp keeps the denominator in [1, inf)
so no row-max pass is needed: overflow to inf gives reciprocal 0, matching
the true underflowed attention weight.

v8 design (cost-model-driven; baseline r1/f32r was 115.5us; this 96.3us,
with PE busy 84.8us = 88% -- the remaining ~11us is the fixed DMA-latency
lead-in and the end-of-kernel exp/recip/scale/DMA/barrier drain):
  - h^T / A / h are prepared HOST-side: pre-transposed, pre-cast to fp16
    (11-bit significand, same as f32r/TF32; PE runs fp16 at 1 cyc/row with
    no moving>=256 constraint).  No on-device transposes or A-rounding.
  - stage 1: g[hd][e, t] = sum_d A[hd][d, e] * hT[d, t], fp16 matmuls into
    [128,512] PSUM, DVE-copied to fp16 g in a HEAD-PACKED layout:
    g[e, ec, i, hd, r] groups both heads' rows for 64-row tile-pairs.
  - stage 2 walks 64-row TILE-PAIRS: the stationary operand packs head0's
    and head1's 64 g-rows into one 128-wide matmul, so both heads' scores
    for the same causal window share every moving column.  Causal waste
    drops from sum 128*(i+1) to sum 64*(i+1) moving cols (-1.7us PE), and
    the diag-block DVE work halves.
  - per tile-pair: the diagonal 512-chunk accumulates FIRST in its own
    2-buf PSUM pool; the causal mask of the diag 64-block is added IN PSUM
    by one extra matmul (lhsT=identity, rhs=cmask64) in the same
    accumulation group; the diag is extracted by a small DVE copy +
    multiply-by-diag-indicator + negated reduce (tensor_tensor_reduce
    crashes the device on this toolchain; DVE two-operand ops must read
    SBUF, copy-class ops may read PSUM); its exp fires FIRST so the pool
    slot the next pair needs frees early.
  - non-diag chunks pair up into [128,1024] PSUM pieces with ONE exp +
    accum_out per piece (ACT exp instrs cost 372ns fixed, so fewer/bigger
    exps keep ACT ~53us and prevent the end-of-kernel ACT backlog v3 had);
    the third chunk of the last group borrows the then-idle stage-1 pool.
    PITFALL: chunks of one pair must live in DIFFERENT psum tiles when
    their exps interleave with later chunk matmuls -- the tile framework
    tracks PSUM deps at tile granularity, so matmuls into the second half
    of a shared tile serialize behind the first half's exp (cost ~1-2us).
  - the out = h[t,:]/denom scale runs on the otherwise-idle Pool engine
    (DVE for the last pairs to shorten the tail); h ships host-replicated
    in the 64-row-pair layout so partitions align.
  - schedule: S1 tsl0 for both heads runs dc-major across 4 concurrent
    psum groups (borrowing 2 stage-2 slots) so the serial input-DMA stream
    paces it without PE gaps; then per group k: the 8 tile-pairs of group
    k interleave 1:1 with the 8 S1 units of tsl k+1.

Engine budget per core (cost model): PE ~84.5us (bound: stage1 27.3 +
stage2 56.3 + mask-adds 0.9), ACT ~52, DVE ~41, Pool ~27, DMA ~45.

Sharding: 16 (b, head) pairs across 8 cores -> core c handles b = c // 4,
heads 2*(c%4) and 2*(c%4)+1.
"""

import sys

try:
    import concourse.bass  # noqa: F401
except ImportError:  # pragma: no cover
    sys.path.insert(0, "/opt/trn_rl_repo")

import numpy as np

import concourse.bass as bass  # noqa: F401
import concourse.tile as tile
from concourse import bacc, bass_utils, mybir

B, T, D, H = 2, 2048, 512, 8
NCORES = 8
P = 128
R = 64           # rows per head in a tile-pair
NP = T // R      # 32 tile-pairs
ND = D // P      # 4 contraction chunks
SCH = 512        # score chunk width (one PSUM bank of fp32)
NEG = -60000.0   # fp16-representable mask value; exp(-6e4 + |score|) == 0

f32 = mybir.dt.float32
f16 = mybir.dt.float16

AX = mybir.AxisListType.X
EXP = mybir.ActivationFunctionType.Exp


def build_nc():
    nc = bacc.Bacc("TRN2", target_bir_lowering=False, debug=False)
    # host-prepared layouts (see make_in_maps):
    #   hTd[p, dc, t]   = h[b, t, dc*128+p]             (fp16)
    #   Ad[p, hd, dc, e] = A[hd][dc*128+p, e]           (fp16)
    #   h64d[m, i, dmn] = h[b, 64*i + m%64, dmn]        (fp16, row-replicated)
    hTd = nc.dram_tensor("hTd", [P, ND, T], f16, kind="ExternalInput")
    Ad = nc.dram_tensor("Ad", [P, 2, ND, D], f16, kind="ExternalInput")
    h64d = nc.dram_tensor("h64d", [P, NP, D], f16, kind="ExternalInput")
    cm64d = nc.dram_tensor("cm64d", [P, R], f16, kind="ExternalInput")
    il64d = nc.dram_tensor("il64d", [P, R], f32, kind="ExternalInput")
    identd = nc.dram_tensor("identd", [P, P], f16, kind="ExternalInput")
    out2 = nc.dram_tensor("out2", [2, T, D], f32, kind="ExternalOutput")

    with tile.TileContext(nc) as tc:
        with (
            tc.tile_pool(name="const", bufs=1) as constp,
            tc.tile_pool(name="big", bufs=1) as big,
            tc.tile_pool(name="s1p", bufs=2, space="PSUM") as s1p,
            tc.tile_pool(name="pDp", bufs=2, space="PSUM") as pDp,
            tc.tile_pool(name="s2p", bufs=2, space="PSUM") as s2p,
            tc.tile_pool(name="stats", bufs=16) as stats,
            tc.tile_pool(name="outp", bufs=4) as outp,
        ):
            # mask constants via the Pool SWDGE queue (Pool idles early)
            cmask = constp.tile([P, R], f16)
            nc.gpsimd.dma_start(out=cmask, in_=cm64d[:])
            ilike = constp.tile([P, R], f32)
            nc.gpsimd.dma_start(out=ilike, in_=il64d[:])
            ident = constp.tile([P, P], f16)
            nc.gpsimd.dma_start(out=ident, in_=identd[:])

            hT = big.tile([P, ND, T], f16)
            A16 = big.tile([P, 2, ND, D], f16)
            h64 = big.tile([P, NP, D], f16)
            # g packed per tile-pair: [e%128, ec, pair, head, row]
            g = big.tile([P, ND, NP, 2, R], f16)
            esc = big.tile([P, 3 * SCH], f32)  # discarded exp output scratch

            # first-needed inputs, interleaved per dc so stage 1 can start
            # ~2.9us in and is then paced by the serial DMA stream.  The
            # very first stationary (A head0/dc0/ec0, 128 cols) ships alone
            # so the first Ldweights+matmul aren't gated on a full 512-col
            # transfer.
            nc.sync.dma_start(out=A16[:, 0, 0, 0:P], in_=Ad[:, 0, 0, 0:P])
            nc.sync.dma_start(out=hT[:, 0, 0:SCH], in_=hTd[:, 0, 0:SCH])
            nc.sync.dma_start(out=A16[:, 0, 0, P:D], in_=Ad[:, 0, 0, P:D])
            for dc in range(1, ND):
                nc.sync.dma_start(out=A16[:, 0, dc], in_=Ad[:, 0, dc])
                nc.sync.dma_start(
                    out=hT[:, dc, 0:SCH], in_=hTd[:, dc, 0:SCH]
                )
            for dc in range(ND):
                nc.sync.dma_start(out=A16[:, 1, dc], in_=Ad[:, 1, dc])

            def dma_tsl(tsl):
                lo = tsl * SCH
                for dc in range(ND):
                    nc.sync.dma_start(
                        out=hT[:, dc, lo : lo + SCH],
                        in_=hTd[:, dc, lo : lo + SCH],
                    )

            def dma_h64(k):  # pairs 4k..4k+3
                nc.sync.dma_start(
                    out=h64[:, 4 * k : 4 * k + 4],
                    in_=h64d[:, 4 * k : 4 * k + 4],
                )

            def s1_copy(ps, hd, tsl, ec):
                nc.vector.tensor_copy(
                    g[:, ec, 8 * tsl : 8 * tsl + 8, hd, :], ps[:, :SCH]
                )

            def emit_s1_unit(hd, tsl, ec):
                ts_ = slice(tsl * SCH, (tsl + 1) * SCH)
                ecs = slice(ec * P, (ec + 1) * P)
                ps = s1p.tile([P, SCH], f32, tag="s1")
                for dc in range(ND):
                    nc.tensor.matmul(
                        ps, A16[:, hd, dc, ecs], hT[:, dc, ts_],
                        start=(dc == 0), stop=(dc == ND - 1),
                    )
                s1_copy(ps, hd, tsl, ec)

            def emit_s1_dcmajor(hd):
                # 4 concurrent ec-groups (2 s1p + 2 s2p slots), matmuls
                # ordered dc-major to match input-DMA arrival order; each
                # group's copy is emitted right after its last matmul so
                # copies overlap the remaining rounds
                pss = [
                    s1p.tile([P, SCH], f32, tag="s1", name=f"s1dm{hd}_{e}")
                    for e in range(2)
                ] + [
                    s2p.tile([P, 2 * SCH], f32, tag="s2", name=f"s1dm{hd}_{e + 2}")
                    for e in range(2)
                ]
                for dc in range(ND):
                    for ec in range(ND):
                        ecs = slice(ec * P, (ec + 1) * P)
                        nc.tensor.matmul(
                            pss[ec][:, :SCH],
                            A16[:, hd, dc, ecs], hT[:, dc, 0:SCH],
                            start=(dc == 0), stop=(dc == ND - 1),
                        )
                        if dc == ND - 1:
                            s1_copy(pss[ec], hd, 0, ec)

            def emit_pair(i, grp):
                W = R * (i + 1)          # causal width of this tile-pair
                nch = (W + SCH - 1) // SCH
                jlast = nch - 1
                wl = W - SCH * jlast     # width of last (diagonal) chunk
                dcol = wl - R            # diag block start within last chunk
                lp = stats.tile([P, 4], f32, tag="lp")

                # diagonal chunk first: its scores feed the bias every exp
                # needs.  The causal mask for the diag 64-block is added in
                # PSUM by one more matmul in the same accumulation group
                # (strictly-upper -6e4; the diagonal itself is 0 so the
                # diag extraction below is unaffected).
                # group 0 pairs have no non-diag chunks, so s2p is idle
                # there: alternate diag chunks between both pools to deepen
                # the effective ring and avoid waiting on the diag exp
                if grp == 0 and i % 2 == 1:
                    pD = s2p.tile([P, 2 * SCH], f32, tag="s2",
                                  name=f"pDs2_{i}")
                else:
                    pD = pDp.tile([P, SCH], f32, tag="pd")
                for ec in range(ND):
                    nc.tensor.matmul(
                        pD[:, :wl],
                        g[:, ec, i],
                        hT[:, ec, jlast * SCH : jlast * SCH + wl],
                        start=(ec == 0), stop=False,
                    )
                nc.tensor.matmul(
                    pD[:, dcol : dcol + R], ident, cmask,
                    start=False, stop=True,
                )
                dblk = stats.tile([P, R], f32, tag="dblk")
                nc.vector.tensor_copy(dblk, pD[:, dcol : dcol + R])
                dmul = stats.tile([P, R], f32, tag="dmul")
                nc.vector.tensor_mul(dmul, dblk, ilike)
                negdiag = stats.tile([P, 1], f32, tag="nd")
                nc.vector.reduce_sum(out=negdiag, in_=dmul, axis=AX, negate=True)
                # diag exp first so the 2-buf pD pool slot frees early
                nc.scalar.activation(
                    out=esc[:, :wl], in_=pD[:, :wl], func=EXP,
                    bias=negdiag, scale=1.0,
                    accum_out=lp[:, 0:1],
                )

                # non-diag chunks pair into [128,1024] pieces, one exp each;
                # a third chunk (last group only) borrows the idle s1 pool.
                # NOTE: a pair's chunks must live in DIFFERENT tiles when
                # their exps interleave with later chunk matmuls -- the tile
                # framework tracks PSUM deps at tile granularity, so matmuls
                # into the second half of a shared tile serialize behind the
                # first half's exp.
                js = list(range(nch - 1))
                pieces = []
                if len(js) >= 1:
                    pieces.append(
                        (s2p.tile([P, 2 * SCH], f32, tag="s2",
                                  name=f"pc_{grp}_{i}_0"), 0, js[:2])
                    )
                if len(js) == 3:
                    pieces.append(
                        (s1p.tile([P, SCH], f32, tag="s1",
                                  name=f"pc_{grp}_{i}_1"), 0, js[2:])
                    )
                for pidx, (pc, base, pjs) in enumerate(pieces):
                    for j in pjs:
                        off = base + SCH * (j - pjs[0])
                        for ec in range(ND):
                            nc.tensor.matmul(
                                pc[:, off : off + SCH],
                                g[:, ec, i],
                                hT[:, ec, j * SCH : (j + 1) * SCH],
                                start=(ec == 0), stop=(ec == ND - 1),
                            )
                    we = SCH * len(pjs)
                    nc.scalar.activation(
                        out=esc[:, :we], in_=pc[:, base : base + we],
                        func=EXP, bias=negdiag, scale=1.0,
                        accum_out=lp[:, 1 + pidx : 2 + pidx],
                    )

                rl = stats.tile([P, 1], f32, tag="rl")
                nacc = 1 + len(pieces)
                if nacc == 1:
                    nc.vector.reciprocal(rl, lp[:, 0:1])
                else:
                    lsum = stats.tile([P, 1], f32, tag="ls")
                    nc.vector.reduce_sum(out=lsum, in_=lp[:, :nacc], axis=AX)
                    nc.vector.reciprocal(rl, lsum)
                ot = outp.tile([P, D], f32, tag="ot")
                if i >= NP - 2:
                    # shorten the tail: DVE beats Pool's launch overhead,
                    # and per-head scale halves let the first out-DMA's
                    # ~1.1us issue latency overlap the second half's scale
                    for hd in range(2):
                        hs = slice(hd * R, (hd + 1) * R)
                        nc.vector.tensor_scalar_mul(
                            ot[hs, :], h64[hs, i], rl[hs, :]
                        )
                        nc.sync.dma_start(
                            out=out2[hd, i * R : (i + 1) * R, :],
                            in_=ot[hs, :],
                        )
                else:
                    nc.gpsimd.tensor_scalar_mul(ot, h64[:, i], rl)
                    for hd in range(2):
                        nc.sync.dma_start(
                            out=out2[hd, i * R : (i + 1) * R, :],
                            in_=ot[hd * R : (hd + 1) * R, :],
                        )

            # ---- schedule ----
            emit_s1_dcmajor(0)
            dma_tsl(1)
            dma_h64(0)
            dma_h64(1)
            emit_s1_dcmajor(1)
            for grp in range(4):
                if grp + 2 <= 3:
                    dma_tsl(grp + 2)
                for k in (2 * grp + 2, 2 * grp + 3):
                    if k < 8:
                        dma_h64(k)
                filler = (
                    [(hd, grp + 1, ec) for hd in range(2) for ec in range(ND)]
                    if grp + 1 <= 3 else []
                )
                for idx, i in enumerate(range(8 * grp, 8 * grp + 8)):
                    if idx < len(filler):
                        emit_s1_unit(*filler[idx])
                    emit_pair(i, grp)

    nc.compile()
    return nc


_NC_CACHE = {}


def _get_nc():
    if "nc" not in _NC_CACHE:
        _NC_CACHE["nc"] = build_nc()
    return _NC_CACHE["nc"]


def _consts():
    r = np.arange(R)
    m = np.arange(P) % R
    cm64 = np.where(r[None, :] > m[:, None], np.float16(NEG), np.float16(0.0))
    il64 = (r[None, :] == m[:, None]).astype(np.float32)
    ident = np.eye(P, dtype=np.float16)
    return cm64.astype(np.float16), il64, ident


def make_in_maps(h, A):
    h = np.ascontiguousarray(h, dtype=np.float32)
    A = np.ascontiguousarray(A, dtype=np.float32)
    cm64, il64, ident = _consts()
    in_maps = []
    for c in range(NCORES):
        b = c // 4
        h0 = 2 * (c % 4)
        hb = h[b]  # [T, D]
        hT = np.ascontiguousarray(
            hb.T.astype(np.float16).reshape(ND, P, T).transpose(1, 0, 2)
        )
        Ah = np.ascontiguousarray(
            A[h0 : h0 + 2].astype(np.float16)
            .reshape(2, ND, P, D).transpose(2, 0, 1, 3)
        )
        h64 = np.ascontiguousarray(
            np.tile(hb.astype(np.float16).reshape(NP, R, D), (1, 2, 1))
            .transpose(1, 0, 2)
        )
        in_maps.append({
            "hTd": hT, "Ad": Ah, "h64d": h64,
            "cm64d": cm64, "il64d": il64, "identd": ident,
        })
    return in_maps


def assemble(results):
    full = np.empty((B, H, T, D), dtype=np.float32)
    for c in range(NCORES):
        b = c // 4
        h0 = 2 * (c % 4)
        o = results[c]["out2"]
        full[b, h0] = o[0]
        full[b, h0 + 1] = o[1]
    return full.reshape(B, T, H * D)


def kernel(h, A):
    nc = _get_nc()
    res = bass_utils.run_bass_kernel_spmd(
        nc, make_in_maps(h, A), core_ids=list(range(NCORES))
    )
    return assemble(res.results)


# revision 33
# speedup vs baseline: 1.0099x; 1.0099x over previous
"""Trainium2 Bass kernel for causal bilinear self-attention (diagonal variant).

Computes, per (b, head):
    scores[t, s] = h[b, t] @ A[head] @ h[b, s]        (causal: s <= t)
    attn = softmax(scores, axis=-1)
    out[b, head, t, :] = attn[t, t] * h[b, t, :]
returned reshaped row-major to (B, T, H*d)  (faithful torch .view semantics).

Only the diagonal of the attention matrix is needed:
    attn[t, t] = 1 / sum_{s<=t} exp(scores[t,s] - scores[t,t])
Using bias = -scores[t,t] inside the exp keeps the denominator in [1, inf)
so no row-max pass is needed: overflow to inf gives reciprocal 0, matching
the true underflowed attention weight.

v8 design (cost-model-driven; baseline r1/f32r was 115.5us; this 96.3us,
with PE busy 84.8us = 88% -- the remaining ~11us is the fixed DMA-latency
lead-in and the end-of-kernel exp/recip/scale/DMA/barrier drain):
  - h^T / A / h are prepared HOST-side: pre-transposed, pre-cast to fp16
    (11-bit significand, same as f32r/TF32; PE runs fp16 at 1 cyc/row with
    no moving>=256 constraint).  No on-device transposes or A-rounding.
  - stage 1: g[hd][e, t] = sum_d A[hd][d, e] * hT[d, t], fp16 matmuls into
    [128,512] PSUM, DVE-copied to fp16 g in a HEAD-PACKED layout:
    g[e, ec, i, hd, r] groups both heads' rows for 64-row tile-pairs.
  - stage 2 walks 64-row TILE-PAIRS: the stationary operand packs head0's
    and head1's 64 g-rows into one 128-wide matmul, so both heads' scores
    for the same causal window share every moving column.  Causal waste
    drops from sum 128*(i+1) to sum 64*(i+1) moving cols (-1.7us PE), and
    the diag-block DVE work halves.
  - per tile-pair: the diagonal 512-chunk accumulates FIRST in its own
    2-buf PSUM pool; the causal mask of the diag 64-block is added IN PSUM
    by one extra matmul (lhsT=identity, rhs=cmask64) in the same
    accumulation group; the diag is extracted by a small DVE copy +
    multiply-by-diag-indicator + negated reduce (tensor_tensor_reduce
    crashes the device on this toolchain; DVE two-operand ops must read
    SBUF, copy-class ops may read PSUM); its exp fires FIRST so the pool
    slot the next pair needs frees early.
  - non-diag chunks pair up into [128,1024] PSUM pieces with ONE exp +
    accum_out per piece (ACT exp instrs cost 372ns fixed, so fewer/bigger
    exps keep ACT ~53us and prevent the end-of-kernel ACT backlog v3 had);
    the third chunk of the last group borrows the then-idle stage-1 pool.
    PITFALL: chunks of one pair must live in DIFFERENT psum tiles when
    their exps interleave with later chunk matmuls -- the tile framework
    tracks PSUM deps at tile granularity, so matmuls into the second half
    of a shared tile serialize behind the first half's exp (cost ~1-2us).
  - the out = h[t,:]/denom scale runs on the otherwise-idle Pool engine
    (DVE for the last pairs to shorten the tail); h ships host-replicated
    in the 64-row-pair layout so partitions align.
  - schedule: S1 tsl0 for both heads runs dc-major across 4 concurrent
    psum groups (borrowing 2 stage-2 slots) so the serial input-DMA stream
    paces it without PE gaps; then per group k: the 8 tile-pairs of group
    k interleave 1:1 with the 8 S1 units of tsl k+1.

Engine budget per core (cost model): PE ~84.5us (bound: stage1 27.3 +
stage2 56.3 + mask-adds 0.9), ACT ~52, DVE ~41, Pool ~27, DMA ~45.

Sharding: 16 (b, head) pairs across 8 cores -> core c handles b = c // 4,
heads 2*(c%4) and 2*(c%4)+1.
"""

import sys

try:
    import concourse.bass  # noqa: F401
except ImportError:  # pragma: no cover
    sys.path.insert(0, "/opt/trn_rl_repo")

import numpy as np

import concourse.bass as bass  # noqa: F401
import concourse.tile as tile
from concourse import bacc, bass_utils, mybir

B, T, D, H = 2, 2048, 512, 8
NCORES = 8
P = 128
R = 64           # rows per head in a tile-pair
NP = T // R      # 32 tile-pairs
ND = D // P      # 4 contraction chunks
SCH = 512        # score chunk width (one PSUM bank of fp32)
NEG = -60000.0   # fp16-representable mask value; exp(-6e4 + |score|) == 0

f32 = mybir.dt.float32
f16 = mybir.dt.float16

AX = mybir.AxisListType.X
EXP = mybir.ActivationFunctionType.Exp


def build_nc():
    nc = bacc.Bacc("TRN2", target_bir_lowering=False, debug=False)
    # host-prepared layouts (see make_in_maps):
    #   hTd[p, dc, t]   = h[b, t, dc*128+p]             (fp16)
    #   Ad[p, hd, dc, e] = A[hd][dc*128+p, e]           (fp16)
    #   h64d[m, i, dmn] = h[b, 64*i + m%64, dmn]        (fp16, row-replicated)
    hTd = nc.dram_tensor("hTd", [P, ND, T], f16, kind="ExternalInput")
    Ad = nc.dram_tensor("Ad", [P, 2, ND, D], f16, kind="ExternalInput")
    h64d = nc.dram_tensor("h64d", [P, NP, D], f16, kind="ExternalInput")
    cm64d = nc.dram_tensor("cm64d", [P, R], f16, kind="ExternalInput")
    il64d = nc.dram_tensor("il64d", [P, R], f32, kind="ExternalInput")
    identd = nc.dram_tensor("identd", [P, P], f16, kind="ExternalInput")
    out2 = nc.dram_tensor("out2", [2, T, D], f32, kind="ExternalOutput")

    with tile.TileContext(nc) as tc:
        with (
            tc.tile_pool(name="const", bufs=1) as constp,
            tc.tile_pool(name="big", bufs=1) as big,
            tc.tile_pool(name="s1p", bufs=2, space="PSUM") as s1p,
            tc.tile_pool(name="pDp", bufs=2, space="PSUM") as pDp,
            tc.tile_pool(name="s2p", bufs=2, space="PSUM") as s2p,
            tc.tile_pool(name="stats", bufs=16) as stats,
            tc.tile_pool(name="outp", bufs=4) as outp,
        ):
            # mask constants via the Pool SWDGE queue (Pool idles early)
            cmask = constp.tile([P, R], f16)
            nc.gpsimd.dma_start(out=cmask, in_=cm64d[:])
            ilike = constp.tile([P, R], f32)
            nc.gpsimd.dma_start(out=ilike, in_=il64d[:])
            ident = constp.tile([P, P], f16)
            nc.gpsimd.dma_start(out=ident, in_=identd[:])

            hT = big.tile([P, ND, T], f16)
            A16 = big.tile([P, 2, ND, D], f16)
            h64 = big.tile([P, NP, D], f16)
            # g packed per tile-pair: [e%128, ec, pair, head, row]
            g = big.tile([P, ND, NP, 2, R], f16)
            esc = big.tile([P, 3 * SCH], f32)  # discarded exp output scratch

            # first-needed inputs, interleaved per dc so stage 1 can start
            # ~3us in and is then paced by the serial DMA stream.  (Tested
            # and rejected: splitting the first A transfer starts the first
            # matmul ~0.35us earlier but shifts every later transfer and
            # adds a semaphore edge -- net +1us.)
            for dc in range(ND):
                nc.sync.dma_start(out=A16[:, 0, dc], in_=Ad[:, 0, dc])
                nc.sync.dma_start(
                    out=hT[:, dc, 0:SCH], in_=hTd[:, dc, 0:SCH]
                )
            for dc in range(ND):
                nc.sync.dma_start(out=A16[:, 1, dc], in_=Ad[:, 1, dc])

            def dma_tsl(tsl):
                lo = tsl * SCH
                for dc in range(ND):
                    nc.sync.dma_start(
                        out=hT[:, dc, lo : lo + SCH],
                        in_=hTd[:, dc, lo : lo + SCH],
                    )

            def dma_h64(k):  # pairs 4k..4k+3
                nc.sync.dma_start(
                    out=h64[:, 4 * k : 4 * k + 4],
                    in_=h64d[:, 4 * k : 4 * k + 4],
                )

            def s1_copy(ps, hd, tsl, ec):
                nc.vector.tensor_copy(
                    g[:, ec, 8 * tsl : 8 * tsl + 8, hd, :], ps[:, :SCH]
                )

            def emit_s1_unit(hd, tsl, ec):
                ts_ = slice(tsl * SCH, (tsl + 1) * SCH)
                ecs = slice(ec * P, (ec + 1) * P)
                ps = s1p.tile([P, SCH], f32, tag="s1")
                for dc in range(ND):
                    nc.tensor.matmul(
                        ps, A16[:, hd, dc, ecs], hT[:, dc, ts_],
                        start=(dc == 0), stop=(dc == ND - 1),
                    )
                s1_copy(ps, hd, tsl, ec)

            def emit_s1_dcmajor(hd):
                # 4 concurrent ec-groups (2 s1p + 2 s2p slots), matmuls
                # ordered dc-major to match input-DMA arrival order; each
                # group's copy is emitted right after its last matmul so
                # copies overlap the remaining rounds
                pss = [
                    s1p.tile([P, SCH], f32, tag="s1", name=f"s1dm{hd}_{e}")
                    for e in range(2)
                ] + [
                    s2p.tile([P, 2 * SCH], f32, tag="s2", name=f"s1dm{hd}_{e + 2}")
                    for e in range(2)
                ]
                for dc in range(ND):
                    for ec in range(ND):
                        ecs = slice(ec * P, (ec + 1) * P)
                        nc.tensor.matmul(
                            pss[ec][:, :SCH],
                            A16[:, hd, dc, ecs], hT[:, dc, 0:SCH],
                            start=(dc == 0), stop=(dc == ND - 1),
                        )
                        if dc == ND - 1:
                            s1_copy(pss[ec], hd, 0, ec)

            def emit_pair(i, grp):
                W = R * (i + 1)          # causal width of this tile-pair
                nch = (W + SCH - 1) // SCH
                jlast = nch - 1
                wl = W - SCH * jlast     # width of last (diagonal) chunk
                dcol = wl - R            # diag block start within last chunk
                lp = stats.tile([P, 4], f32, tag="lp")

                # diagonal chunk first: its scores feed the bias every exp
                # needs.  The causal mask for the diag 64-block is added in
                # PSUM by one more matmul in the same accumulation group
                # (strictly-upper -6e4; the diagonal itself is 0 so the
                # diag extraction below is unaffected).
                # group 0 pairs have no non-diag chunks, so s2p is idle
                # there: alternate diag chunks between both pools to deepen
                # the effective ring and avoid waiting on the diag exp
                if grp == 0 and i % 2 == 1:
                    pD = s2p.tile([P, 2 * SCH], f32, tag="s2",
                                  name=f"pDs2_{i}")
                else:
                    pD = pDp.tile([P, SCH], f32, tag="pd")
                for ec in range(ND):
                    nc.tensor.matmul(
                        pD[:, :wl],
                        g[:, ec, i],
                        hT[:, ec, jlast * SCH : jlast * SCH + wl],
                        start=(ec == 0), stop=False,
                    )
                nc.tensor.matmul(
                    pD[:, dcol : dcol + R], ident, cmask,
                    start=False, stop=True,
                )
                dblk = stats.tile([P, R], f32, tag="dblk")
                nc.vector.tensor_copy(dblk, pD[:, dcol : dcol + R])
                dmul = stats.tile([P, R], f32, tag="dmul")
                nc.vector.tensor_mul(dmul, dblk, ilike)
                negdiag = stats.tile([P, 1], f32, tag="nd")
                nc.vector.reduce_sum(out=negdiag, in_=dmul, axis=AX, negate=True)
                # diag exp first so the 2-buf pD pool slot frees early
                nc.scalar.activation(
                    out=esc[:, :wl], in_=pD[:, :wl], func=EXP,
                    bias=negdiag, scale=1.0,
                    accum_out=lp[:, 0:1],
                )

                # non-diag chunks pair into [128,1024] pieces, one exp each;
                # a third chunk (last group only) borrows the idle s1 pool.
                # NOTE: a pair's chunks must live in DIFFERENT tiles when
                # their exps interleave with later chunk matmuls -- the tile
                # framework tracks PSUM deps at tile granularity, so matmuls
                # into the second half of a shared tile serialize behind the
                # first half's exp.
                js = list(range(nch - 1))
                pieces = []
                if len(js) >= 1:
                    pieces.append(
                        (s2p.tile([P, 2 * SCH], f32, tag="s2",
                                  name=f"pc_{grp}_{i}_0"), 0, js[:2])
                    )
                if len(js) == 3:
                    pieces.append(
                        (s1p.tile([P, SCH], f32, tag="s1",
                                  name=f"pc_{grp}_{i}_1"), 0, js[2:])
                    )
                for pidx, (pc, base, pjs) in enumerate(pieces):
                    for j in pjs:
                        off = base + SCH * (j - pjs[0])
                        for ec in range(ND):
                            nc.tensor.matmul(
                                pc[:, off : off + SCH],
                                g[:, ec, i],
                                hT[:, ec, j * SCH : (j + 1) * SCH],
                                start=(ec == 0), stop=(ec == ND - 1),
                            )
                    we = SCH * len(pjs)
                    nc.scalar.activation(
                        out=esc[:, :we], in_=pc[:, base : base + we],
                        func=EXP, bias=negdiag, scale=1.0,
                        accum_out=lp[:, 1 + pidx : 2 + pidx],
                    )

                rl = stats.tile([P, 1], f32, tag="rl")
                nacc = 1 + len(pieces)
                if nacc == 1:
                    nc.vector.reciprocal(rl, lp[:, 0:1])
                else:
                    lsum = stats.tile([P, 1], f32, tag="ls")
                    nc.vector.reduce_sum(out=lsum, in_=lp[:, :nacc], axis=AX)
                    nc.vector.reciprocal(rl, lsum)
                ot = outp.tile([P, D], f32, tag="ot")
                if i >= NP - 2:
                    # shorten the tail: DVE beats Pool's launch overhead,
                    # and per-head scale halves let the first out-DMA's
                    # ~1.1us issue latency overlap the second half's scale
                    for hd in range(2):
                        hs = slice(hd * R, (hd + 1) * R)
                        nc.vector.tensor_scalar_mul(
                            ot[hs, :], h64[hs, i], rl[hs, :]
                        )
                        nc.sync.dma_start(
                            out=out2[hd, i * R : (i + 1) * R, :],
                            in_=ot[hs, :],
                        )
                else:
                    nc.gpsimd.tensor_scalar_mul(ot, h64[:, i], rl)
                    for hd in range(2):
                        nc.sync.dma_start(
                            out=out2[hd, i * R : (i + 1) * R, :],
                            in_=ot[hd * R : (hd + 1) * R, :],
                        )

            # ---- schedule ----
            emit_s1_dcmajor(0)
            dma_tsl(1)
            dma_h64(0)
            dma_h64(1)
            emit_s1_dcmajor(1)
            for grp in range(4):
                if grp + 2 <= 3:
                    dma_tsl(grp + 2)
                for k in (2 * grp + 2, 2 * grp + 3):
                    if k < 8:
                        dma_h64(k)
                filler = (
                    [(hd, grp + 1, ec) for hd in range(2) for ec in range(ND)]
                    if grp + 1 <= 3 else []
                )
                for idx, i in enumerate(range(8 * grp, 8 * grp + 8)):
                    if idx < len(filler):
                        emit_s1_unit(*filler[idx])
                    emit_pair(i, grp)

    nc.compile()
    return nc


_NC_CACHE = {}


def _get_nc():
    if "nc" not in _NC_CACHE:
        _NC_CACHE["nc"] = build_nc()
    return _NC_CACHE["nc"]


def _consts():
    r = np.arange(R)
    m = np.arange(P) % R
    cm64 = np.where(r[None, :] > m[:, None], np.float16(NEG), np.float16(0.0))
    il64 = (r[None, :] == m[:, None]).astype(np.float32)
    ident = np.eye(P, dtype=np.float16)
    return cm64.astype(np.float16), il64, ident


def make_in_maps(h, A):
    h = np.ascontiguousarray(h, dtype=np.float32)
    A = np.ascontiguousarray(A, dtype=np.float32)
    cm64, il64, ident = _consts()
    in_maps = []
    for c in range(NCORES):
        b = c // 4
        h0 = 2 * (c % 4)
        hb = h[b]  # [T, D]
        hT = np.ascontiguousarray(
            hb.T.astype(np.float16).reshape(ND, P, T).transpose(1, 0, 2)
        )
        Ah = np.ascontiguousarray(
            A[h0 : h0 + 2].astype(np.float16)
            .reshape(2, ND, P, D).transpose(2, 0, 1, 3)
        )
        h64 = np.ascontiguousarray(
            np.tile(hb.astype(np.float16).reshape(NP, R, D), (1, 2, 1))
            .transpose(1, 0, 2)
        )
        in_maps.append({
            "hTd": hT, "Ad": Ah, "h64d": h64,
            "cm64d": cm64, "il64d": il64, "identd": ident,
        })
    return in_maps


def assemble(results):
    full = np.empty((B, H, T, D), dtype=np.float32)
    for c in range(NCORES):
        b = c // 4
        h0 = 2 * (c % 4)
        o = results[c]["out2"]
        full[b, h0] = o[0]
        full[b, h0 + 1] = o[1]
    return full.reshape(B, T, H * D)


def kernel(h, A):
    nc = _get_nc()
    res = bass_utils.run_bass_kernel_spmd(
        nc, make_in_maps(h, A), core_ids=list(range(NCORES))
    )
    return assemble(res.results)
